# revision 15
# baseline (speedup 1.0000x reference)
"""Trainium2 Bass kernel for nn_CrossAttention (sparse_attention).

Math: the reference's softmax is over a size-1 axis -> attn == 1.0
everywhere, and cross_x = v = k = x[j ^ 1] @ W_qkv.T.  So the device
work is a single batched matmul with a pair-swap on the batch axis.

Sharding: data-parallel over batch, 64 samples (32 complete pairs) per
core.  The pair swap and the [tokens, C] -> [C, tokens] transpose are
folded into host-side shard prep; the device streams:

    xt  [128, 25088]  fp32   (partitions = 2 token-halves x 64 channels)
    wst [128, 128]    fp32   (W_qkv.T stacked twice on partitions)
    yt  [128, 50176]  fp32   (partitions = D; columns = tokens)

Per core: 7 iterations x (1 load [128,3584], 14 K=64 matmuls N=512,
14 PSUM->SBUF copies split across DVE/ACT, 2 stores [128,3584]).
"""

import sys

for _p in ("/root/.axon_site", "/root/.axon_site/_ro/trn_rl_repo",
           "/root/.axon_site/_ro/pypackages", "/opt/trn_rl_repo"):
    if _p not in sys.path:
        sys.path.append(_p)

import numpy as np

import concourse.bass as bass
import concourse.mybir as mybir
from concourse.bacc import Bacc
from concourse.bass_utils import run_bass_kernel_spmd
from concourse.tile import TileContext

B = 512
HW = 784
C = 64
D = 128
N_CORES = 8
SAMPLES_PER_CORE = B // N_CORES          # 64
T = SAMPLES_PER_CORE * HW                # 50176 tokens per core
T_HALF = T // 2                          # 25088
COLS = 3584                              # tokens per load/store tile
N_ITERS = T_HALF // COLS                 # 7
MM_N = 512                               # moving free dim per matmul
MM_PER_COL = COLS // MM_N                # 7

USE_F32R = False                         # fp32r matmul: 1 cyc/row vs 4 for fp32


def _build_nc(t_half: int = T_HALF, cols: int = COLS, mm_n: int = MM_N,
              use_f32r: bool = USE_F32R) -> bass.Bass:
    n_iters = t_half // cols
    mm_per_col = cols // mm_n
    assert t_half % cols == 0 and cols % mm_n == 0

    nc = Bacc()
    f32 = mybir.dt.float32
    xt = nc.dram_tensor("xt", [128, t_half], f32, kind="ExternalInput")
    wst = nc.dram_tensor("wst", [128, D], f32, kind="ExternalInput")
    yt = nc.dram_tensor("yt", [D, 2 * t_half], f32, kind="ExternalOutput")

    mm_dt = mybir.dt.float32r if use_f32r else f32

    with TileContext(nc) as tc:
        with (
            tc.tile_pool(name="wpool", bufs=1) as wpool,
            tc.tile_pool(name="xpool", bufs=5) as xpool,
            tc.tile_pool(name="spool", bufs=3) as spool,
            tc.tile_pool(name="psum", bufs=8, space="PSUM") as psum,
        ):
            w_tile = wpool.tile([128, D], f32)
            nc.sync.dma_start(out=w_tile[:, :], in_=wst[:, :])

            for i in range(n_iters):
                x_tile = xpool.tile([128, cols], f32)
                nc.sync.dma_start(out=x_tile[:, :],
                                  in_=xt[:, i * cols:(i + 1) * cols])
                # interleave the two 64-row halves: adjacent matmuls hit
                # disjoint PE row-groups and execute concurrently
                stages = [spool.tile([128, cols], f32, tag=f"stage{h}",
                                     name=f"stage{h}_{i}")
                          for h in range(2)]
                for j in range(mm_per_col):
                    for h in range(2):
                        pt = psum.tile([128, mm_n], f32)
                        lhsT = w_tile[64 * h:64 * h + 64, :].bitcast(mm_dt)
                        rhs = x_tile[64 * h:64 * h + 64,
                                     j * mm_n:(j + 1) * mm_n].bitcast(mm_dt)
                        nc.tensor.matmul(pt[:, :], lhsT, rhs,
                                         start=True, stop=True)
                        cp = (nc.vector.tensor_copy if h == 0
                              else nc.scalar.copy)
                        cp(stages[h][:, j * mm_n:(j + 1) * mm_n], pt[:, :])
                for h in range(2):
                    nc.scalar.dma_start(
                        out=yt[:, h * t_half + i * cols:
                               h * t_half + (i + 1) * cols],
                        in_=stages[h][:, :])
    return nc


_NC_CACHE: dict = {}


def _get_nc(**kw) -> bass.Bass:
    key = tuple(sorted(kw.items()))
    if key not in _NC_CACHE:
        nc = _build_nc(**kw)
        nc.finalize()   # Bacc: splits multi-sem waits, allocates registers
        _NC_CACHE[key] = nc
    return _NC_CACHE[key]


def _prep_shard(x: np.ndarray, s: int) -> np.ndarray:
    """[128, T_HALF] fp32: rows h*64+c hold channel c of token-half h,
    with the pair swap (out[j] needs x[j^1]) already applied."""
    idx = 64 * s + (np.arange(SAMPLES_PER_CORE) ^ 1)
    xs = x[idx]                                    # [64, 784, 64] copy
    v = xs.reshape(2, T_HALF, C).transpose(0, 2, 1)  # view [2, 64, 25088]
    return np.ascontiguousarray(v).reshape(128, T_HALF)


def kernel(x: np.ndarray, W_qkv: np.ndarray):
    x = np.asarray(x, dtype=np.float32)
    W_qkv = np.asarray(W_qkv, dtype=np.float32)
    assert x.shape == (B, HW, C) and W_qkv.shape == (D, C)

    wstack = np.ascontiguousarray(
        np.concatenate([W_qkv.T, W_qkv.T], axis=0))   # [128, 128]

    in_maps = [{"xt": _prep_shard(x, s), "wst": wstack}
               for s in range(N_CORES)]

    nc = _get_nc()
    try:
        res = run_bass_kernel_spmd(nc, in_maps,
                                   core_ids=list(range(N_CORES)))
    except Exception:
        # transient NRT device errors (wedged core) usually clear on retry
        import time
        time.sleep(2.0)
        res = run_bass_kernel_spmd(nc, in_maps,
                                   core_ids=list(range(N_CORES)))

    cross_x = np.empty((B, HW, D), dtype=np.float32)
    for s in range(N_CORES):
        ys = res.results[s]["yt"]                     # [128, 50176]
        cross_x[64 * s:64 * (s + 1)] = (
            ys.reshape(D, SAMPLES_PER_CORE, HW).transpose(1, 2, 0))

    attn = np.ones((B, HW, 1, 1), dtype=np.float32)
    return cross_x, attn


# revision 16
# speedup vs baseline: 1.0035x; 1.0035x over previous
"""Trainium2 Bass kernel for nn_CrossAttention (sparse_attention).

Math: the reference's softmax is over a size-1 axis -> attn == 1.0
everywhere, and cross_x = v = k = x[j ^ 1] @ W_qkv.T.  So the device
work is a single batched matmul with a pair-swap on the batch axis.

Sharding: data-parallel over batch, 64 samples (32 complete pairs) per
core.  The pair swap and the [tokens, C] -> [C, tokens] transpose are
folded into host-side shard prep; the device streams:

    xt  [128, 25088]  fp32   (partitions = 2 token-halves x 64 channels)
    wst [128, 128]    fp32   (W_qkv.T stacked twice on partitions)
    yt  [128, 50176]  fp32   (partitions = D; columns = tokens)

Per core: 7 iterations x (1 load [128,3584], 14 K=64 matmuls N=512,
14 PSUM->SBUF copies split across DVE/ACT, 2 stores [128,3584]).
"""

import sys

for _p in ("/root/.axon_site", "/root/.axon_site/_ro/trn_rl_repo",
           "/root/.axon_site/_ro/pypackages", "/opt/trn_rl_repo"):
    if _p not in sys.path:
        sys.path.append(_p)

import numpy as np

import concourse.bass as bass
import concourse.mybir as mybir
from concourse.bacc import Bacc
from concourse.bass_utils import run_bass_kernel_spmd
from concourse.tile import TileContext

B = 512
HW = 784
C = 64
D = 128
N_CORES = 8
SAMPLES_PER_CORE = B // N_CORES          # 64
T = SAMPLES_PER_CORE * HW                # 50176 tokens per core
T_HALF = T // 2                          # 25088
COLS = 3584                              # tokens per load/store tile
N_ITERS = T_HALF // COLS                 # 7
MM_N = 512                               # moving free dim per matmul
MM_PER_COL = COLS // MM_N                # 7

USE_F32R = False                         # fp32r matmul: 1 cyc/row vs 4 for fp32


def _build_nc(t_half: int = T_HALF, cols: int = COLS, mm_n: int = MM_N,
              use_f32r: bool = USE_F32R) -> bass.Bass:
    n_iters = t_half // cols
    mm_per_col = cols // mm_n
    assert t_half % cols == 0 and cols % mm_n == 0

    nc = Bacc()
    f32 = mybir.dt.float32
    xt = nc.dram_tensor("xt", [128, t_half], f32, kind="ExternalInput")
    wst = nc.dram_tensor("wst", [128, D], f32, kind="ExternalInput")
    yt = nc.dram_tensor("yt", [D, 2 * t_half], f32, kind="ExternalOutput")

    mm_dt = mybir.dt.float32r if use_f32r else f32

    with TileContext(nc) as tc:
        with (
            tc.tile_pool(name="wpool", bufs=1) as wpool,
            tc.tile_pool(name="xpool", bufs=5) as xpool,
            tc.tile_pool(name="spool", bufs=2) as spool,
            tc.tile_pool(name="psum", bufs=8, space="PSUM") as psum,
        ):
            w_tile = wpool.tile([128, D], f32)
            nc.sync.dma_start(out=w_tile[:, :], in_=wst[:, :])

            for i in range(n_iters):
                x_tile = xpool.tile([128, cols], f32)
                nc.sync.dma_start(out=x_tile[:, :],
                                  in_=xt[:, i * cols:(i + 1) * cols])
                # interleave the two 64-row halves: adjacent matmuls hit
                # disjoint PE row-groups and execute concurrently
                stages = [spool.tile([128, cols], f32, tag=f"stage{h}",
                                     name=f"stage{h}_{i}")
                          for h in range(2)]
                for j in range(mm_per_col):
                    for h in range(2):
                        pt = psum.tile([128, mm_n], f32)
                        lhsT = w_tile[64 * h:64 * h + 64, :].bitcast(mm_dt)
                        rhs = x_tile[64 * h:64 * h + 64,
                                     j * mm_n:(j + 1) * mm_n].bitcast(mm_dt)
                        nc.tensor.matmul(pt[:, :], lhsT, rhs,
                                         start=True, stop=True)
                        cp = (nc.vector.tensor_copy if h == 0
                              else nc.scalar.copy)
                        cp(stages[h][:, j * mm_n:(j + 1) * mm_n], pt[:, :])
                for h in range(2):
                    nc.scalar.dma_start(
                        out=yt[:, h * t_half + i * cols:
                               h * t_half + (i + 1) * cols],
                        in_=stages[h][:, :])
    return nc


_NC_CACHE: dict = {}


def _get_nc(**kw) -> bass.Bass:
    key = tuple(sorted(kw.items()))
    if key not in _NC_CACHE:
        nc = _build_nc(**kw)
        nc.finalize()   # Bacc: splits multi-sem waits, allocates registers
        _NC_CACHE[key] = nc
    return _NC_CACHE[key]


def _prep_shard(x: np.ndarray, s: int) -> np.ndarray:
    """[128, T_HALF] fp32: rows h*64+c hold channel c of token-half h,
    with the pair swap (out[j] needs x[j^1]) already applied."""
    idx = 64 * s + (np.arange(SAMPLES_PER_CORE) ^ 1)
    xs = x[idx]                                    # [64, 784, 64] copy
    v = xs.reshape(2, T_HALF, C).transpose(0, 2, 1)  # view [2, 64, 25088]
    return np.ascontiguousarray(v).reshape(128, T_HALF)


def kernel(x: np.ndarray, W_qkv: np.ndarray):
    x = np.asarray(x, dtype=np.float32)
    W_qkv = np.asarray(W_qkv, dtype=np.float32)
    assert x.shape == (B, HW, C) and W_qkv.shape == (D, C)

    wstack = np.ascontiguousarray(
        np.concatenate([W_qkv.T, W_qkv.T], axis=0))   # [128, 128]

    in_maps = [{"xt": _prep_shard(x, s), "wst": wstack}
               for s in range(N_CORES)]

    nc = _get_nc()
    try:
        res = run_bass_kernel_spmd(nc, in_maps,
                                   core_ids=list(range(N_CORES)))
    except Exception:
        # transient NRT device errors (wedged core) usually clear on retry
        import time
        time.sleep(2.0)
        res = run_bass_kernel_spmd(nc, in_maps,
                                   core_ids=list(range(N_CORES)))

    cross_x = np.empty((B, HW, D), dtype=np.float32)
    for s in range(N_CORES):
        ys = res.results[s]["yt"]                     # [128, 50176]
        cross_x[64 * s:64 * (s + 1)] = (
            ys.reshape(D, SAMPLES_PER_CORE, HW).transpose(1, 2, 0))

    attn = np.ones((B, HW, 1, 1), dtype=np.float32)
    return cross_x, attn


# revision 17
# speedup vs baseline: 1.1886x; 1.1845x over previous
"""Trainium2 Bass kernel for nn_CrossAttention (sparse_attention).

Math: the reference's softmax is over a size-1 axis -> attn == 1.0
everywhere, and cross_x = v = k = x[j ^ 1] @ W_qkv.T.  So the device
work is a single batched matmul with a pair-swap on the batch axis.

Sharding: data-parallel over batch, 64 samples (32 complete pairs) per
core.  The pair swap and the [tokens, C] -> [C, tokens] transpose are
folded into host-side shard prep; the device streams:

    xt  [128, 25088]  fp32   (partitions = 2 token-halves x 64 channels)
    wst [128, 128]    fp32   (W_qkv.T stacked twice on partitions)
    yt  [128, 50176]  fp32   (partitions = D; columns = tokens)

Per core: 7 iterations x (1 load [128,3584], 14 K=64 matmuls N=512,
14 PSUM->SBUF copies split across DVE/ACT, 2 stores [128,3584]).
"""

import sys

for _p in ("/root/.axon_site", "/root/.axon_site/_ro/trn_rl_repo",
           "/root/.axon_site/_ro/pypackages", "/opt/trn_rl_repo"):
    if _p not in sys.path:
        sys.path.append(_p)

import numpy as np

import concourse.bass as bass
import concourse.mybir as mybir
from concourse.bacc import Bacc
from concourse.bass_utils import run_bass_kernel_spmd
from concourse.tile import TileContext

B = 512
HW = 784
C = 64
D = 128
N_CORES = 8
SAMPLES_PER_CORE = B // N_CORES          # 64
T = SAMPLES_PER_CORE * HW                # 50176 tokens per core
T_HALF = T // 2                          # 25088
COLS = 3584                              # tokens per load/store tile
N_ITERS = T_HALF // COLS                 # 7
MM_N = 512                               # moving free dim per matmul
MM_PER_COL = COLS // MM_N                # 7

USE_F32R = False                         # fp32r matmul: 1 cyc/row vs 4 for fp32


def _build_nc(t_half: int = T_HALF, cols: int = COLS, mm_n: int = MM_N,
              use_f32r: bool = USE_F32R) -> bass.Bass:
    n_iters = t_half // cols
    mm_per_col = cols // mm_n
    assert t_half % cols == 0 and cols % mm_n == 0

    nc = Bacc(enable_partition_id=False, monotonic_sem_count=0)
    f32 = mybir.dt.float32
    xt = nc.dram_tensor("xt", [128, t_half], f32, kind="ExternalInput")
    wst = nc.dram_tensor("wst", [128, D], f32, kind="ExternalInput")
    yt = nc.dram_tensor("yt", [D, 2 * t_half], f32, kind="ExternalOutput")

    mm_dt = mybir.dt.float32r if use_f32r else f32

    with TileContext(nc) as tc:
        with (
            tc.tile_pool(name="wpool", bufs=1) as wpool,
            tc.tile_pool(name="xpool", bufs=5) as xpool,
            tc.tile_pool(name="spool", bufs=2) as spool,
            tc.tile_pool(name="psum", bufs=8, space="PSUM") as psum,
        ):
            w_tile = wpool.tile([128, D], f32)
            nc.sync.dma_start(out=w_tile[:, :], in_=wst[:, :])

            for i in range(n_iters):
                x_tile = xpool.tile([128, cols], f32)
                nc.sync.dma_start(out=x_tile[:, :],
                                  in_=xt[:, i * cols:(i + 1) * cols])
                # interleave the two 64-row halves: adjacent matmuls hit
                # disjoint PE row-groups and execute concurrently
                stages = [spool.tile([128, cols], f32, tag=f"stage{h}",
                                     name=f"stage{h}_{i}")
                          for h in range(2)]
                for j in range(mm_per_col):
                    for h in range(2):
                        pt = psum.tile([128, mm_n], f32)
                        lhsT = w_tile[64 * h:64 * h + 64, :].bitcast(mm_dt)
                        rhs = x_tile[64 * h:64 * h + 64,
                                     j * mm_n:(j + 1) * mm_n].bitcast(mm_dt)
                        nc.tensor.matmul(pt[:, :], lhsT, rhs,
                                         start=True, stop=True)
                        cp = (nc.vector.tensor_copy if h == 0
                              else nc.scalar.copy)
                        cp(stages[h][:, j * mm_n:(j + 1) * mm_n], pt[:, :])
                for h in range(2):
                    nc.scalar.dma_start(
                        out=yt[:, h * t_half + i * cols:
                               h * t_half + (i + 1) * cols],
                        in_=stages[h][:, :])
    return nc


_NC_CACHE: dict = {}


def _get_nc(**kw) -> bass.Bass:
    key = tuple(sorted(kw.items()))
    if key not in _NC_CACHE:
        nc = _build_nc(**kw)
        nc.finalize()   # Bacc: splits multi-sem waits, allocates registers
        _NC_CACHE[key] = nc
    return _NC_CACHE[key]


def _prep_shard(x: np.ndarray, s: int) -> np.ndarray:
    """[128, T_HALF] fp32: rows h*64+c hold channel c of token-half h,
    with the pair swap (out[j] needs x[j^1]) already applied."""
    idx = 64 * s + (np.arange(SAMPLES_PER_CORE) ^ 1)
    xs = x[idx]                                    # [64, 784, 64] copy
    v = xs.reshape(2, T_HALF, C).transpose(0, 2, 1)  # view [2, 64, 25088]
    return np.ascontiguousarray(v).reshape(128, T_HALF)


def kernel(x: np.ndarray, W_qkv: np.ndarray):
    x = np.asarray(x, dtype=np.float32)
    W_qkv = np.asarray(W_qkv, dtype=np.float32)
    assert x.shape == (B, HW, C) and W_qkv.shape == (D, C)

    wstack = np.ascontiguousarray(
        np.concatenate([W_qkv.T, W_qkv.T], axis=0))   # [128, 128]

    in_maps = [{"xt": _prep_shard(x, s), "wst": wstack}
               for s in range(N_CORES)]

    nc = _get_nc()
    try:
        res = run_bass_kernel_spmd(nc, in_maps,
                                   core_ids=list(range(N_CORES)))
    except Exception:
        # transient NRT device errors (wedged core) usually clear on retry
        import time
        time.sleep(2.0)
        res = run_bass_kernel_spmd(nc, in_maps,
                                   core_ids=list(range(N_CORES)))

    cross_x = np.empty((B, HW, D), dtype=np.float32)
    for s in range(N_CORES):
        ys = res.results[s]["yt"]                     # [128, 50176]
        cross_x[64 * s:64 * (s + 1)] = (
            ys.reshape(D, SAMPLES_PER_CORE, HW).transpose(1, 2, 0))

    attn = np.ones((B, HW, 1, 1), dtype=np.float32)
    return cross_x, attn
